# revision 39
# baseline (speedup 1.0000x reference)
"""Trainium2 Bass kernel for nn_PolicyNetwork (dense_mlp, 8-core data-parallel).

Reference computation:
  op branch:    ops = softmax_global( MLP3([x, y[dag], z]) - 1000*(1-op_msk) )
  prlvl branch: prlvl = softmax_rows( MLP3([limits, y, z]) - 1000*(1-prlvl_msk) )

Sharding: x/op_msk split along rows (12800/core), y/prlvl_msk along dags
(128/core); weights/z replicated.  The global op softmax uses an AllGather of
per-core (max, sumexp) partials.

op branch, per core (window pipeline, 25 windows of 512 rows):
  - x window [128,4,256] -> 8 PE transposes -> xT [128 feat, 512 rows]
  - L1 feature-major in fp32r (1 cyc/row): h1T[32,512] = 2 K-chunk matmuls
    + a K=7 indicator matmul folding the per-dag bias c1 = y@W1y + z@W1z + b1
  - 4 windows pack a [128,512] PSUM bank (quarters); plain ACT relu
  - L2/L3 use block-diagonal quad weights: one matmul per window-quad,
    L3 emits [4,512] logit rows; mask fused via scalar_tensor_tensor
    (the -1000 mask constant is softmax-invariant and dropped)
  - masked softmax on [25,512]; cross-core AllGather of (max, sumexp)
  - b3 is softmax-invariant in both branches -> dropped
"""

import numpy as np

import concourse.bass as bass
import concourse.tile as tile
from concourse import bacc, bass_isa, mybir
from concourse.bass_utils import run_bass_kernel_spmd

FP = mybir.dt.float32
FR = mybir.dt.float32r
E = 256
D_TOT = 1024
W_WORKERS = 64
OPS = 100
H1, H2 = 32, 16
N_CORES = 8
DC = D_TOT // N_CORES           # 128 dags per core
ROWS_C = DC * OPS               # 12800 rows per core
N_WIN = ROWS_C // 512           # 25 windows of 512 rows
N_TILE = (N_WIN + 3) // 4       # 7 window-quads


def emit(tc, io, n_cores):
    from contextlib import ExitStack
    ctx = ExitStack()
    nc = tc.nc
    dc = DC
    nj = dc // 4
    nj2 = dc // 8

    cpool = ctx.enter_context(tc.tile_pool(name="consts", bufs=1))
    dpool = ctx.enter_context(tc.tile_pool(name="dram", bufs=1, space="DRAM"))

    def cload(name, shape, src_ap=None):
        t = cpool.tile(shape, FP, tag=name)
        nc.sync.dma_start(t[:], io[name] if src_ap is None else src_ap)
        return t

    ident = cload("ident", [128, 128])
    identr = cpool.tile([128, 128], FR, tag="identr")
    nc.sync.dma_start(identr[:], io["ident"].bitcast(FR))
    ones1 = cload("ones1", [1, 128])

    # first-layer weights: [256,32] -> [128, 2x32] (chunk k at cols 32k)
    def wchunks(name, dt=FP):
        t = cpool.tile([128, 64], dt, tag=name)
        nc.sync.dma_start(t[:].rearrange("p (k h) -> p k h", k=2),
                          io[name].bitcast(dt).rearrange("(k p) h -> p k h", k=2))
        return t

    w1x = wchunks("op_w1x", FR)
    w1y = wchunks("op_w1y")
    w1yp = wchunks("pr_w1y")
    czop = cload("czb1op", [1, 32])
    czpr = cload("czb1pr", [1, 32])
    w2qd = cpool.tile([128, 64], FR, tag="w2qd")
    nc.sync.dma_start(w2qd[:], io["op_w2quad"].bitcast(FR))
    w3qd = cpool.tile([64, 4], FR, tag="w3qd")
    nc.sync.dma_start(w3qd[:], io["op_w3quad"].bitcast(FR))
    b2qd = cload("op_b2quad", [64, 1])
    ind7 = cpool.tile([7, ROWS_C], FR, tag="ind7")
    nc.sync.dma_start(ind7[:], io["ind7"].bitcast(FR))
    w2pb = cload("pr_w2blk", [128, 64])
    w3pb = cload("pr_w3blk", [128, 8])
    b2pt = cload("pr_b2tile", [128, 1])
    lwp = cload("lwpack", [128, 64])
    y_sb = cload("y_sh", [dc, E])
    prm = cload("prm_sh", [dc, W_WORKERS])

    spool = ctx.enter_context(tc.tile_pool(name="sb", bufs=1))

    # ---- yT + dag-bias vectors c1 = y@W1y + (z@W1z + b1) -------------------
    with tc.tile_pool(name="ps_pre", bufs=1, space="PSUM") as psa:
        ypt = psa.tile([128, 2 * dc], FP, tag="ypt")
        for k in range(2):
            nc.tensor.matmul(ypt[:, dc * k:dc * k + dc],
                             y_sb[:, 128 * k:128 * k + 128],
                             ident[:dc, :dc], is_transpose=True,
                             start=(k == 0), stop=(k == 1),
                             skip_group_check=True)
        yts = spool.tile([128, 2 * dc], FP, tag="yts")
        nc.scalar.copy(yts[:], ypt[:])

        c1_sb = {}
        for nm, wch, cz in (("op", w1y, czop), ("pr", w1yp, czpr)):
            c1p = psa.tile([dc, 32], FP, tag=f"c1{nm}p")
            for k in range(2):
                nc.tensor.matmul(c1p[:], yts[:, dc * k:dc * k + dc],
                                 wch[:, 32 * k:32 * k + 32],
                                 start=(k == 0), stop=False)
            nc.tensor.matmul(c1p[:], ones1[:1, :dc], cz[:],
                             start=False, stop=True)
            sb = spool.tile([dc, 32], FP, tag=f"c1{nm}sb")
            nc.scalar.copy(sb[:], c1p[:])
            c1_sb[nm] = sb

    # DRAM bounces: c1pr packed for prlvl relu bias; c1op windowed for the
    # L1 bias matmul (c1lin7[:, 32w:32w+32] = c1op rows dag0(w)..dag0(w)+7)
    bncs = {}
    for nm in ("op", "pr"):
        bnc = dpool.tile([dc, 32], FP, tag=f"bnc_{nm}")
        nc.sync.dma_start(bnc[:], c1_sb[nm][:])
        bncs[nm] = bnc
    c1prpack = spool.tile([128, nj], FP, tag="c1prpack")
    nc.sync.dma_start(c1prpack[:],
                      bncs["pr"][:].rearrange("(j g) h -> (g h) j", g=4))
    c1lin7 = spool.tile([7, 32 * N_WIN], FR, tag="c1lin7")
    nc.vector.memset(c1lin7[:].bitcast(FP), 0.0)
    for w in range(N_WIN):
        dag0 = (512 * w) // OPS
        rcount = min(7, dc - dag0)
        nc.sync.dma_start(c1lin7[0:rcount, 32 * w:32 * w + 32],
                          bncs["op"][dag0:dag0 + rcount, :].bitcast(FR))

    # ---- prlvl branch ------------------------------------------------------
    with tc.tile_pool(name="ps_pr", bufs=2, space="PSUM") as psp, \
         tc.tile_pool(name="ps_prl", bufs=1, space="PSUM") as pspl:
        h1cat = []
        for j in range(nj):
            if j % 4 == 0:
                h1cat.append(spool.tile([128, 256], FP, name="h1cat",
                                        tag=f"h1cat{(j // 4) % 4}"))
            nc.vector.tensor_scalar_add(
                h1cat[-1][:, 64 * (j % 4):64 * (j % 4) + 64], lwp[:],
                c1prpack[:, j:j + 1])
        h1r = []
        for t in range(nj // 4):
            r = spool.tile([128, 256], FP, tag=f"h1r{t % 4}")
            nc.scalar.activation(r[:], h1cat[t][:],
                                 mybir.ActivationFunctionType.Relu)
            h1r.append(r)

        def h1p(j):
            return h1r[j // 4][:, 64 * (j % 4):64 * (j % 4) + 64]

        h2strip = []
        for t in range((nj2 + 7) // 8):
            l2pp = psp.tile([128, 512], FP, tag="l2pp")
            for s in range(min(8, nj2 - 8 * t)):
                j2 = 8 * t + s
                for jj in range(2):
                    nc.tensor.matmul(
                        l2pp[64 * jj:64 * jj + 64, 64 * s:64 * s + 64],
                        w2pb[:], h1p(2 * j2 + jj),
                        start=(s == 0), stop=(s == min(8, nj2 - 8 * t) - 1),
                        skip_group_check=True)
            r = spool.tile([128, 512], FP, tag=f"h2strip{t % 2}")
            nc.scalar.activation(r[:], l2pp[:],
                                 mybir.ActivationFunctionType.Relu,
                                 bias=b2pt[:, 0:1])
            h2strip.append(r)

        prmadj = spool.tile([dc, W_WORKERS], FP, tag="prmadj")
        nc.vector.tensor_scalar_mul(prmadj[:], prm[:], 1000.0)
        lpT = pspl.tile([W_WORKERS, dc], FP, tag="lpT")
        for j2 in range(nj2):
            nc.tensor.matmul(lpT[:, 8 * j2:8 * j2 + 8],
                             h2strip[j2 // 8][:, 64 * (j2 % 8):64 * (j2 % 8) + 64],
                             w3pb[:], start=(j2 == 0), stop=False,
                             skip_group_check=True)
        nc.tensor.matmul(lpT[:], prmadj[:], ident[:dc, :dc],
                         start=False, stop=True, skip_group_check=True)
        lpts = spool.tile([W_WORKERS, dc], FP, tag="lpts")
        nc.scalar.copy(lpts[:], lpT[:])
        lpm = psp.tile([dc, W_WORKERS], FP, tag="l2pp")
        nc.tensor.matmul(lpm[:], lpts[:], ident[:W_WORKERS, :W_WORKERS],
                         start=True, stop=True)
        nmx = spool.tile([dc, 1], FP, tag="nmx")
        nc.vector.tensor_reduce(nmx[:], lpm[:], axis=mybir.AxisListType.X,
                                op=mybir.AluOpType.max, negate=True)
        ep = spool.tile([dc, W_WORKERS], FP, tag="ep")
        sep = spool.tile([dc, 1], FP, tag="sep")
        nc.scalar.activation(ep[:], lpm[:], mybir.ActivationFunctionType.Exp,
                             bias=nmx[:, 0:1], accum_out=sep[:])
        rp = spool.tile([dc, 1], FP, tag="rp")
        nc.vector.reciprocal(rp[:], sep[:])
        pro = spool.tile([dc, W_WORKERS], FP, tag="pro")
        nc.vector.tensor_scalar_mul(pro[:], ep[:], rp[:, 0:1])
        nc.sync.dma_start(io["prlvl_out"], pro[:])

    # ---- op branch: per-window pipeline ------------------------------------
    x_re = io["x_sh"].rearrange("(w jj p) e -> w p jj e", jj=4, p=128)
    opm_re = io["opm_sh"].rearrange("(p f) -> p f", p=N_WIN)
    lm25 = spool.tile([N_WIN, 512], FP, tag="lm25")

    with tc.tile_pool(name="ps_xt", bufs=1, space="PSUM") as psxt, \
         tc.tile_pool(name="xin", bufs=3) as xpool, \
         tc.tile_pool(name="xts", bufs=3) as xtpool, \
         tc.tile_pool(name="ps_l1", bufs=4, space="PSUM") as psl1, \
         tc.tile_pool(name="ps_h2", bufs=1, space="PSUM") as psh2, \
         tc.tile_pool(name="ps_lg", bufs=1, space="PSUM") as pslg, \
         tc.tile_pool(name="h12", bufs=2) as hpool, \
         tc.tile_pool(name="lgp", bufs=2) as gpool:
        for t in range(N_TILE):
            nwt = min(4, N_WIN - 4 * t)
            h1t = hpool.tile([128, 512], FR, tag="h1t")
            for wq in range(nwt):
                w = 4 * t + wq
                x4 = xpool.tile([128, 4, E], FR, tag="x4")
                nc.sync.dma_start(x4[:], x_re[w].bitcast(FR))
                xtp = psxt.tile([128, 1024], FR, tag="xtp")
                for jj in range(4):
                    for k in range(2):
                        nc.tensor.matmul(
                            xtp[:, 512 * k + 128 * jj:512 * k + 128 * jj + 128],
                            x4[:, jj, 128 * k:128 * k + 128], identr[:],
                            is_transpose=True, start=(jj == 0),
                            stop=(jj == 3 and k == 1), skip_group_check=True)
                xt = xtpool.tile([128, 1024], FR, tag="xt")
                nc.vector.tensor_copy(xt[:, 0:512], xtp[:, 0:512])
                nc.scalar.copy(xt[:, 512:1024], xtp[:, 512:1024])
                l1w = psl1.tile([32, 512], FP, tag="l1w")
                for k in range(2):
                    nc.tensor.matmul(l1w[:], w1x[:, 32 * k:32 * k + 32],
                                     xt[:, 512 * k:512 * k + 512],
                                     start=(k == 0), stop=False,
                                     skip_group_check=True)
                nc.tensor.matmul(l1w[:], c1lin7[:, 32 * w:32 * w + 32],
                                 ind7[:, 512 * w:512 * w + 512],
                                 start=False, stop=True, skip_group_check=True)
                hq = h1t[32 * wq:32 * wq + 32, :]
                if wq % 2 == 0:
                    nc.vector.tensor_relu(hq, l1w[:])
                else:
                    nc.scalar.activation(hq, l1w[:],
                                         mybir.ActivationFunctionType.Relu)
            h2ps = psh2.tile([64, 512], FP, tag="h2ps")
            nc.tensor.matmul(h2ps[0:16 * nwt, :],
                             w2qd[0:32 * nwt, 0:16 * nwt],
                             h1t[0:32 * nwt, :],
                             start=True, stop=True, skip_group_check=True)
            h2t = hpool.tile([64, 512], FR, tag="h2t")
            nc.scalar.activation(h2t[0:16 * nwt, :], h2ps[0:16 * nwt, :],
                                 mybir.ActivationFunctionType.Relu,
                                 bias=b2qd[:16 * nwt, 0:1])
            lg4 = pslg.tile([4, 512], FP, tag="lg4")
            nc.tensor.matmul(lg4[0:nwt, :], w3qd[0:16 * nwt, 0:nwt],
                             h2t[0:16 * nwt, :],
                             start=True, stop=True, skip_group_check=True)
            opm4 = gpool.tile([4, 512], FP, tag="opm4")
            nc.sync.dma_start(opm4[0:nwt, :], opm_re[4 * t:4 * t + nwt, :])
            lgm = gpool.tile([4, 512], FP, tag="lgm")
            nc.vector.scalar_tensor_tensor(lgm[0:nwt, :], opm4[0:nwt, :],
                                           1000.0, lg4[0:nwt, :],
                                           op0=mybir.AluOpType.mult,
                                           op1=mybir.AluOpType.add)
            nc.sync.dma_start(lm25[4 * t:4 * t + nwt, :], lgm[0:nwt, :])

        # ---- masked global softmax on [N_WIN, 512] -------------------------
        lm = lm25
        mx = spool.tile([N_WIN, 1], FP, tag="mx")
        nc.vector.tensor_reduce(mx[:], lm[:], axis=mybir.AxisListType.X,
                                op=mybir.AluOpType.max)
        mcb = spool.tile([N_WIN, 1], FP, tag="mcb")
        nc.gpsimd.partition_all_reduce(mcb[:], mx[:], N_WIN,
                                       bass_isa.ReduceOp.max)
        mcnb = spool.tile([N_WIN, 1], FP, tag="mcnb")
        nc.vector.tensor_scalar_mul(mcnb[:], mcb[:], -1.0)
        eo = spool.tile([N_WIN, 512], FP, tag="eo")
        se = spool.tile([N_WIN, 1], FP, tag="se")
        nc.scalar.activation(eo[:], lm[:], mybir.ActivationFunctionType.Exp,
                             bias=mcnb[:, 0:1], accum_out=se[:])
        scb = spool.tile([N_WIN, 1], FP, tag="scb")
        nc.gpsimd.partition_all_reduce(scb[:], se[:], N_WIN,
                                       bass_isa.ReduceOp.add)

        part = spool.tile([1, 2], FP, tag="part")
        nc.vector.tensor_copy(part[0:1, 0:1], mcb[0:1, 0:1])
        nc.vector.tensor_copy(part[0:1, 1:2], scb[0:1, 0:1])
        ag_in = dpool.tile([1, 2], FP, tag="ag_in")
        ag_out = dpool.tile([1, 2 * n_cores], FP, tag="ag_out")
        nc.sync.dma_start(ag_in[:], part[:])
        nc.gpsimd.collective_compute(
            "AllGather", mybir.AluOpType.bypass,
            replica_groups=[list(range(n_cores))],
            ins=[ag_in.opt()], outs=[ag_out.opt()])
        gth = spool.tile([1, 2 * n_cores], FP, tag="gth")
        nc.sync.dma_start(gth[:], ag_out[:])

        gv = gth[:].rearrange("p (c t) -> p t c", t=2)  # [1, 2, n_cores]
        nM = spool.tile([1, 1], FP, tag="nM")
        nc.vector.tensor_reduce(nM[:], gv[:, 0:1, :],
                                axis=mybir.AxisListType.X,
                                op=mybir.AluOpType.max, negate=True)
        em = spool.tile([1, n_cores], FP, tag="em")
        nc.scalar.activation(em[:], gv[:, 0, :],
                             mybir.ActivationFunctionType.Exp,
                             bias=nM[:, 0:1])
        zs = spool.tile([1, n_cores], FP, tag="zs")
        nc.vector.tensor_mul(zs[:], em[:], gv[:, 1, :])
        zt = spool.tile([1, 1], FP, tag="zt")
        nc.vector.tensor_reduce(zt[:], zs[:], axis=mybir.AxisListType.X,
                                op=mybir.AluOpType.add)
        am = spool.tile([1, 1], FP, tag="am")
        nc.scalar.activation(am[:], mcb[0:1, 0:1],
                             mybir.ActivationFunctionType.Exp,
                             bias=nM[:, 0:1])
        zi = spool.tile([1, 1], FP, tag="zi")
        nc.vector.reciprocal(zi[:], zt[:])
        alpha = spool.tile([1, 1], FP, tag="alpha")
        nc.vector.tensor_mul(alpha[:], am[:], zi[:])
        alphab = spool.tile([N_WIN, 1], FP, tag="alphab")
        nc.gpsimd.partition_broadcast(alphab[:], alpha[:])
        oout = spool.tile([N_WIN, 512], FP, tag="oout")
        nc.vector.tensor_scalar_mul(oout[:], eo[:], alphab[:, 0:1])
        nc.sync.dma_start(io["ops_out"].rearrange("(p f) -> p f", p=N_WIN),
                          oout[:])
    ctx.close()


def build(n_cores=N_CORES):
    nc = bacc.Bacc("TRN2", target_bir_lowering=False, debug=False,
                   num_devices=n_cores)
    names_in = {
        "ident": [128, 128], "ones1": [1, 128],
        "op_w1x": [E, H1], "op_w1y": [E, H1], "pr_w1y": [E, H1],
        "czb1op": [1, H1], "czb1pr": [1, H1],
        "op_w2quad": [128, 64], "op_w3quad": [64, 4],
        "op_b2quad": [64, 1], "ind7": [7, ROWS_C],
        "pr_w2blk": [128, 64], "pr_w3blk": [128, 8], "pr_b2tile": [128, 1],
        "lwpack": [128, W_WORKERS],
        "x_sh": [ROWS_C, E], "y_sh": [DC, E],
        "opm_sh": [ROWS_C], "prm_sh": [DC, W_WORKERS],
    }
    io = {}
    for nm, shp in names_in.items():
        io[nm] = nc.dram_tensor(nm, shp, FP, kind="ExternalInput").ap()
    io["ops_out"] = nc.dram_tensor("ops_out", [ROWS_C], FP,
                                   kind="ExternalOutput").ap()
    io["prlvl_out"] = nc.dram_tensor("prlvl_out", [DC, W_WORKERS], FP,
                                     kind="ExternalOutput").ap()
    with tile.TileContext(nc) as tc:
        emit(tc, io, n_cores)
    nc.compile()
    return nc


def make_const_inputs(op_W1, op_b1, op_W2, op_b2, op_W3,
                      pr_W1, pr_b1, pr_W2, pr_b2, pr_W3, z):
    """Host-side constant prep (tiny numpy)."""
    f = np.float32

    def blk4(w2):  # [32,16] -> [128,64] block-diag x4 (prlvl L2)
        out = np.zeros((128, 64), f)
        for g in range(4):
            out[32 * g:32 * g + 32, 16 * g:16 * g + 16] = w2
        return out

    def blk8(w3):  # [16,1] -> [128,8]; partition (jj,g,h2) -> dag 4jj+g
        out = np.zeros((128, 8), f)
        for p in range(128):
            jj, g, h2 = p // 64, (p % 64) // 16, p % 16
            out[p, 4 * jj + g] = w3[h2, 0]
        return out

    w3quad = np.zeros((64, 4), f)
    for q in range(4):
        w3quad[16 * q:16 * q + 16, q] = np.asarray(op_W3, f)[:, 0]

    ind7 = np.zeros((7, ROWS_C), f)
    for w in range(N_WIN):
        base = 512 * w
        dag0 = base // OPS
        for c in range(512):
            ind7[(base + c) // OPS - dag0, base + c] = 1.0

    lw = np.outer(pr_W1[0], np.arange(1, W_WORKERS + 1, dtype=f))  # [32,64]
    return {
        "ident": np.eye(128, dtype=f),
        "ones1": np.ones((1, 128), f),
        "op_w1x": np.ascontiguousarray(op_W1[0:E], f),
        "op_w1y": np.ascontiguousarray(op_W1[E:2 * E], f),
        "pr_w1y": np.ascontiguousarray(pr_W1[1:1 + E], f),
        "czb1op": (z[0] @ op_W1[2 * E:3 * E] + op_b1).reshape(1, H1).astype(f),
        "czb1pr": (z[0] @ pr_W1[1 + E:1 + 2 * E] + pr_b1).reshape(1, H1).astype(f),
        "op_w2quad": blk4(np.asarray(op_W2, f)),
        "op_w3quad": w3quad,
        "op_b2quad": np.tile(np.asarray(op_b2, f), 4).reshape(64, 1),
        "ind7": ind7,
        "pr_w2blk": blk4(np.asarray(pr_W2, f)),
        "pr_w3blk": blk8(np.asarray(pr_W3, f)),
        "pr_b2tile": np.tile(np.asarray(pr_b2, f), 8).reshape(128, 1),
        "lwpack": np.tile(lw, (4, 1)).astype(f),
    }


_compiled = {}
TRACE = False           # set by test harness to capture NTFF profile
LAST_RESULTS = None     # BassKernelResults of the last kernel() run


def _get_compiled():
    if "full" not in _compiled:
        _compiled["full"] = build()
    return _compiled["full"]


def kernel(num_ops, num_dags, num_workers, x, y, z, op_msk, prlvl_msk,
           op_W1, op_b1, op_W2, op_b2, op_W3, op_b3,
           pr_W1, pr_b1, pr_W2, pr_b2, pr_W3, pr_b3):
    x = np.asarray(x, np.float32)
    y = np.asarray(y, np.float32)
    z = np.asarray(z, np.float32)
    op_msk = np.asarray(op_msk, np.float32)
    prlvl_msk = np.asarray(prlvl_msk, np.float32)
    consts = make_const_inputs(
        np.asarray(op_W1), np.asarray(op_b1), np.asarray(op_W2),
        np.asarray(op_b2), np.asarray(op_W3),
        np.asarray(pr_W1), np.asarray(pr_b1), np.asarray(pr_W2),
        np.asarray(pr_b2), np.asarray(pr_W3), z)

    nc = _get_compiled()
    in_maps = []
    for c in range(N_CORES):
        m = dict(consts)
        m["x_sh"] = x[c * ROWS_C:(c + 1) * ROWS_C]
        m["y_sh"] = y[c * DC:(c + 1) * DC]
        m["opm_sh"] = op_msk[c * ROWS_C:(c + 1) * ROWS_C]
        m["prm_sh"] = prlvl_msk[c * DC:(c + 1) * DC]
        in_maps.append(m)

    global LAST_RESULTS
    res = run_bass_kernel_spmd(nc, in_maps, list(range(N_CORES)),
                               trace=TRACE)
    LAST_RESULTS = res
    ops = np.concatenate(
        [res.results[c]["ops_out"] for c in range(N_CORES)])
    prlvl = np.concatenate(
        [res.results[c]["prlvl_out"] for c in range(N_CORES)], axis=0)
    return ops, prlvl
